# revision 17
# baseline (speedup 1.0000x reference)
"""Trainium2 Bass kernel for nn_AdaptivePruner (moe_routing, 8 NeuronCores).

Strategy
--------
Pure data parallel: batch 32 split 4 samples/core across 8 cores. The
routing statistics (mean/std of CLS-attention entropy over the full
batch) are needed by every core, but the stats input is tiny (32x4096
f32 = 512 KB), so instead of an all-reduce each core receives the FULL
cls_attention_map (rotated so its own 4 samples sit at rows 0..3 --
mean/std are permutation invariant) and computes the stats redundantly.
No collectives.

Per-core compute, all on-chip:
  * entropy: ACT ln + DVE multiply/reduce -> ent[32]
  * stats: DVE 32x32 transpose -> mean / unbiased var on one partition;
    level-2 gate in the squared domain (no sqrt):
      m2 = (ent<mean) & ((ent-mean)^2 > 2.25*var) & (var>1e-12), m1=1-m2
    broadcast to 128 partitions (gpsimd partition_broadcast).
  * expert select folded into the WEIGHTS: per sample the conv1 weights
    are scaled by m1 and the composite-y2 weights by m2 (exact: m is 0/1),
    so both experts accumulate into ONE PSUM tile and the epilogue is a
    single PSUM->SBUF copy.
  * wavelets as banded matmuls (sequence dim on partitions, D=768 on the
    free axis split 2x384 to fit PSUM banks), in float32r (single-pass
    PE mode, ~1.6e-4 rel err):
      y1 = db4 lowpass stride 2 (8 taps): 3 matmuls / 128-row chunk
      y2 = cascaded lowpass DIRECTLY from x via the composite 22-tap
           stride-4 filter g = (h upsampled by 2) conv h, exactly equal
           to dwt_lo(dwt_lo(x)) incl. zero-padding: 5 matmuls / chunk
  * loads on the sync (HWDGE) ring, stores + cls stats load on the
    gpsimd (SWDGE) ring so stores never head-of-line-block loads.

HBM traffic/core ~76 MB -> ~220 us roofline at ~350 GB/s/core;
PE ~170 us, DVE+ACT ~45 us each, hidden under DMA.
"""
import numpy as np

import concourse.bacc as bacc
import concourse.mybir as mybir
import concourse.tile as tile
from concourse.bass_utils import run_bass_kernel_spmd

AF = mybir.ActivationFunctionType
OP = mybir.AluOpType
DT = mybir.dt

B, NTOK, D = 32, 4097, 768
NPAT, L1, L2 = 4096, 2051, 1029
NCORES, SPC = 8, 4
NCH = 17          # output chunks per sample: 16 x 128 rows + 3-row tail
H = 2             # free-dim halves (2 x 384 <= one PSUM bank each)
HD = D // H

DEC_LO = np.array([-0.010597401785069032, 0.0328830116668852,
                   0.030841381835560764, -0.18703481171909309,
                   -0.027983769416859854, 0.6308807679298589,
                   0.7148465705529157, 0.2303778133088965], np.float64)
K8 = DEC_LO[::-1].copy()
G22 = np.zeros(22, np.float64)
for _j in range(8):
    for _k in range(8):
        G22[2 * _j + _k] += K8[_j] * K8[_k]

# runtime knobs (test harness pokes these)
TRACE = False
LAST_RESULT = None


def _band(filt, S, OFF, Dn, R=128):
    """W[m, t] = filt[128*Dn + m - S*t + OFF] (0 outside filter support)."""
    W = np.zeros((128, R), np.float32)
    m = np.arange(128)
    for t in range(R):
        idx = 128 * Dn + m - S * t + OFF
        ok = (idx >= 0) & (idx < len(filt))
        W[ok, t] = filt[idx[ok]]
    return W


def _weights():
    wc1 = np.concatenate([_band(K8, 2, 6, dn) for dn in (-1, 0, 1)], axis=1)
    wy2 = np.concatenate([_band(G22, 4, 18, dn) for dn in (-1, 0, 1, 2, 3)],
                         axis=1)
    return np.ascontiguousarray(wc1), np.ascontiguousarray(wy2)


def _build():
    import os
    mode = os.environ.get("KMODE", "full")  # full | conv (forced lvl1)
    nc = bacc.Bacc(None, target_bir_lowering=False)
    x = nc.declare_dram_parameter("x", [SPC, NTOK, D], DT.float32r,
                                  isOutput=False)
    cls = nc.declare_dram_parameter("cls", [B, NPAT], DT.float32,
                                    isOutput=False)
    wc1 = nc.declare_dram_parameter("wc1", [128, 3 * 128], DT.float32r,
                                    isOutput=False)
    wy2 = nc.declare_dram_parameter("wy2", [128, 5 * 128], DT.float32r,
                                    isOutput=False)
    out = nc.declare_dram_parameter("out", [SPC, 1 + L1, D], DT.float32,
                                    isOutput=True)
    mout = nc.declare_dram_parameter("mout", [2, B], DT.float32,
                                     isOutput=True)
    ones = nc.declare_dram_parameter("ones", [1, 128], DT.float32r,
                                     isOutput=False)

    with tile.TileContext(nc) as tc:
        with (
            tc.tile_pool(name="wp", bufs=1) as wp,
            tc.tile_pool(name="ws", bufs=2) as wsp,
            tc.tile_pool(name="ent", bufs=1) as entp,
            tc.tile_pool(name="xin", bufs=6) as xin,
            tc.tile_pool(name="ob", bufs=4) as obp,
            tc.tile_pool(name="otail", bufs=2) as otp,
            tc.tile_pool(name="crow", bufs=2) as crp,
            tc.tile_pool(name="tmp", bufs=2) as tmpp,
            tc.tile_pool(name="pa", bufs=4, space="PSUM") as pap,
        ):
            # ---------------- raw weights ----------------
            wc1t = wp.tile([128, 3 * 128], DT.float32r)
            wy2t = wp.tile([128, 5 * 128], DT.float32r)
            onest = wp.tile([1, 128], DT.float32r)

            # ---------------- entropy + routing stats ----------------
            mcols = entp.tile([128, 2 * B], DT.float32)
            if mode == "conv":
                nc.sync.dma_start(wc1t[:], wc1[:, :])
                nc.sync.dma_start(wy2t[:], wy2[:, :])
                nc.sync.dma_start(onest[:], ones[:, :])
                nc.vector.memset(mcols[:, 0:B], 1.0)
                nc.vector.memset(mcols[:, B:2 * B], 0.0)
            else:
                # cls reshaped to [128, 1024]: partition 4*b + q holds
                # quarter q of sample b -> 4x faster ACT/DVE big ops.
                # Loaded as the FIRST sync-ring DMA so it drains before the
                # x batches (FIFO per HWDGE ring).
                ent128 = entp.tile([128, NPAT // 4], DT.float32)
                csrc = cls[:, :].rearrange("b (q j) -> b q j", q=4)
                nc.sync.dma_start(ent128[:], csrc)
                nc.sync.dma_start(wc1t[:], wc1[:, :])
                nc.sync.dma_start(wy2t[:], wy2[:, :])
                nc.sync.dma_start(onest[:], ones[:, :])
                lnp = entp.tile([128, NPAT // 4], DT.float32)
                bias9 = entp.tile([128, 1], DT.float32)
                nc.vector.memset(bias9[:], 1e-9)
                # dependency-free warm-up Ln so the ACT table set loads
                # while the cls DMA is still in flight
                warm = entp.tile([128, 1], DT.float32)
                nc.scalar.activation(warm[:], bias9[:], AF.Ln, bias=bias9[:])
                nc.scalar.activation(lnp[:], ent128[:], AF.Ln, bias=bias9[:])
                # fused (-1/ln2 * p) * ln(p+1e-9) + row-sum -> [128, 1]
                scr = entp.tile([128, NPAT // 4], DT.float32)
                part = entp.tile([128, 1], DT.float32)
                nc.vector.scalar_tensor_tensor(scr[:], ent128[:],
                                               -1.4426950408889634,
                                               lnp[:], op0=OP.mult,
                                               op1=OP.mult,
                                               accum_out=part[:])
                # partition -> free flatten via a tiny DMA, then sum the 4
                # quarters per sample along the free axis
                prow = entp.tile([1, 128], DT.float32)
                nc.scalar.dma_start(prow[:], part[:])
                entrow = entp.tile([1, 32], DT.float32)
                prview = prow[0:1, :].rearrange("o (b q) -> o b q", q=4)
                nc.vector.tensor_reduce(entrow[:], prview,
                                        axis=mybir.AxisListType.X, op=OP.add)
                erow = entrow[0:1, 0:B]
                ssum = entp.tile([1, 1], DT.float32)
                nc.vector.tensor_reduce(ssum[:], erow,
                                        axis=mybir.AxisListType.X, op=OP.add)
                mean = entp.tile([1, 1], DT.float32)
                nc.vector.tensor_scalar_mul(mean[:], ssum[:], 1.0 / B)
                dev = entp.tile([1, B], DT.float32)
                nc.vector.tensor_scalar_sub(dev[:], erow, mean[0:1, 0:1])
                dev2 = entp.tile([1, B], DT.float32)
                nc.vector.tensor_mul(dev2[:], dev[:], dev[:])
                ss2 = entp.tile([1, 1], DT.float32)
                nc.vector.tensor_reduce(ss2[:], dev2[:],
                                        axis=mybir.AxisListType.X, op=OP.add)
                var = entp.tile([1, 1], DT.float32)
                nc.vector.tensor_scalar_mul(var[:], ss2[:], 1.0 / (B - 1))
                var225 = entp.tile([1, 1], DT.float32)
                nc.vector.tensor_scalar_mul(var225[:], var[:], 2.25)
                c1 = entp.tile([1, B], DT.float32)
                nc.vector.tensor_scalar(c1[:], dev[:], 0.0, None,
                                        op0=OP.is_lt)
                c2 = entp.tile([1, B], DT.float32)
                nc.vector.tensor_scalar(c2[:], dev2[:], var225[0:1, 0:1],
                                        None, op0=OP.is_gt)
                c3 = entp.tile([1, 1], DT.float32)
                nc.vector.tensor_scalar(c3[:], var[:], 1e-12, None,
                                        op0=OP.is_gt)
                m2a = entp.tile([1, B], DT.float32)
                nc.vector.tensor_mul(m2a[:], c1[:], c2[:])
                mpack = entp.tile([1, 2 * B], DT.float32r)
                m2row = entp.tile([1, B], DT.float32)
                nc.vector.tensor_scalar_mul(m2row[:], m2a[:], c3[0:1, 0:1])
                nc.vector.tensor_scalar_mul(
                    mpack[:, B:2 * B], m2row[:], 1.0)
                nc.vector.tensor_scalar(mpack[:, 0:B], m2row[:], -1.0, 1.0,
                                        op0=OP.mult, op1=OP.add)
                # exact broadcast to 128 partitions via PE: ones^T @ mpack
                mps = pap.tile([128, 2 * B], DT.float32, tag="pa")
                nc.tensor.matmul(mps[:], onest[:], mpack[:],
                                 start=True, stop=True)
                nc.vector.tensor_copy(mcols[:], mps[:])
                nc.gpsimd.dma_start(mout[0:1, :], m2row[:])
                nc.gpsimd.dma_start(mout[1:2, :], entrow[0:1, 0:B])

            # ---------------- main per-sample pipeline ----------------
            for b in range(SPC):
                m1c = mcols[:, b:b + 1]
                m2c = mcols[:, B + b:B + b + 1]

                # expert select folded into the weights (m is exactly 0/1)
                # for b >= 1; sample 0 runs both experts unscaled into two
                # PSUM tiles and selects in the epilogue, so its matmuls
                # need only the raw weights (PE starts ~15us earlier)
                fused = b > 0
                if fused:
                    wc1s = wsp.tile([128, 3 * 128], DT.float32r, tag="wc1s")
                    nc.scalar.mul(wc1s[:], wc1t[:], m1c)
                    wy2s = wsp.tile([128, 5 * 128], DT.float32r, tag="wy2s")
                    nc.vector.tensor_scalar_mul(wy2s[:], wy2t[:], m2c)
                else:
                    wc1s, wy2s = wc1t, wy2t

                # CLS token passthrough
                crow = crp.tile([1, D], DT.float32r)
                nc.sync.dma_start(crow[:], x[b, 0:1, :])
                nc.gpsimd.dma_start(out[b, 0:1, :],
                                    crow[:].bitcast(DT.float32))

                # patch loads: 8 DMAs of 4 chunks each
                xts = []
                for q in range(8):
                    xt = xin.tile([128, 4, D], DT.float32r)
                    src = x[b, 1 + 512 * q: 1 + 512 * (q + 1), :].rearrange(
                        "(blk p) d -> p blk d", p=128)
                    nc.sync.dma_start(xt[:], src)
                    xts.append(xt)

                def xch(i, h):
                    q, r = divmod(i, 4)
                    return xts[q][:, r, h * HD:(h + 1) * HD]

                ob = None
                for jc in range(NCH):
                    R = 128 if jc < 16 else 3
                    qo, blk = divmod(jc, 4)
                    # matmul schedule for this chunk: conv1 and y2;
                    # fused: all into one PSUM tile (weights pre-scaled)
                    # unfused (b==0): separate PSUM tiles + epilogue select
                    s1, s2 = [], []
                    if jc < 16:
                        for dn in (-1, 0, 1):
                            if 0 <= 2 * jc + dn < 32:
                                s1.append((wc1s, dn, 2 * jc + dn, R))
                    else:
                        s1.append((wc1s, -1, 31, R))
                    if jc < 8:
                        for dn in (-1, 0, 1, 2, 3):
                            if 0 <= 4 * jc + dn < 32:
                                s2.append((wy2s, dn, 4 * jc + dn, 128))
                    elif jc == 8:
                        # tail y2 rows 0..4; full-128 weight slice: columns
                        # past t=4 are all-zero so rows 5..127 get +0
                        s2.append((wy2s, -1, 31, 128))

                    pa = pap.tile([128, H, 512], DT.float32)
                    pb = None
                    if fused:
                        groups = [(pa, s1 + s2)]
                    else:
                        groups = [(pa, s1)]
                        if s2:
                            pb = pap.tile([128, H, 512], DT.float32,
                                          tag="pa")
                            groups.append((pb, s2))
                    for ps, sched in groups:
                        for h in range(H):
                            for k, (wt, dn, i, r) in enumerate(sched):
                                nc.tensor.matmul(
                                    ps[0:r, h, 0:HD],
                                    wt[:, (dn + 1) * 128:(dn + 1) * 128 + r],
                                    xch(i, h),
                                    start=(k == 0),
                                    stop=(k == len(sched) - 1))

                    # epilogue: PSUM->SBUF (fused: plain copy; sample 0:
                    # final = m1*PA + m2*PB select), alternating engines
                    if jc < 16:
                        if blk == 0:
                            ob = obp.tile([128, 4, H, HD], DT.float32)
                        dst = ob[0:R, blk]
                    else:
                        otail = otp.tile([3, H, HD], DT.float32)
                        dst = otail[0:R]
                    if fused:
                        if jc % 2:
                            nc.vector.tensor_copy(dst, pa[0:R, :, 0:HD])
                        else:
                            nc.scalar.copy(dst, pa[0:R, :, 0:HD])
                    elif pb is not None:
                        tmp = tmpp.tile([128, H, HD], DT.float32)
                        nc.scalar.mul(tmp[0:R], pb[0:R, :, 0:HD], m2c[0:R])
                        nc.vector.scalar_tensor_tensor(
                            dst, pa[0:R, :, 0:HD], m1c[0:R], tmp[0:R],
                            op0=OP.mult, op1=OP.add)
                    else:
                        if jc % 2:
                            nc.vector.tensor_scalar_mul(
                                dst, pa[0:R, :, 0:HD], m1c[0:R])
                        else:
                            nc.scalar.mul(dst, pa[0:R, :, 0:HD], m1c[0:R])

                    # flush output batches (SWDGE ring; never blocks loads)
                    if jc < 16 and blk == 3:
                        d_ap = out[b, 1 + 512 * qo: 1 + 512 * (qo + 1),
                                   :].rearrange("(blk p) d -> p blk d", p=128)
                        nc.gpsimd.dma_start(d_ap, ob[:])
                    elif jc == 16:
                        nc.gpsimd.dma_start(out[b, 2049:2052, :], dst)
    nc.compile()
    return nc


_NC = None


def _get_nc():
    global _NC
    if _NC is None:
        _NC = _build()
    return _NC


def kernel(x, cls_attention_map):
    global LAST_RESULT
    x = np.ascontiguousarray(np.asarray(x, dtype=np.float32))
    cls = np.ascontiguousarray(np.asarray(cls_attention_map,
                                          dtype=np.float32))
    wc1, wy2 = _weights()
    in_maps = []
    for c in range(NCORES):
        in_maps.append({
            "x": np.ascontiguousarray(x[SPC * c: SPC * (c + 1)]),
            "cls": np.ascontiguousarray(np.roll(cls, -SPC * c, axis=0)),
            "wc1": wc1,
            "wy2": wy2,
            "ones": np.ones((1, 128), np.float32),
        })
    res = run_bass_kernel_spmd(_get_nc(), in_maps,
                               core_ids=list(range(NCORES)), trace=TRACE)
    LAST_RESULT = res

    fin = np.empty((B, 1 + L1, D), np.float32)
    lvl = np.empty(B, np.int64)
    for c in range(NCORES):
        r = res.results[c]
        fin[SPC * c: SPC * (c + 1)] = r["out"]
        m2v = np.asarray(r["mout"]).reshape(2, B)[0, 0:SPC]
        lvl[SPC * c: SPC * (c + 1)] = 1 + (m2v > 0.5).astype(np.int64)
    out_len = np.where(lvl == 1, L1, L2)
    mask = np.arange(1 + L1)[None, :] < (1 + out_len)[:, None]
    return fin, mask


# revision 18
# speedup vs baseline: 1.1379x; 1.1379x over previous
"""Trainium2 Bass kernel for nn_AdaptivePruner (moe_routing, 8 NeuronCores).

Strategy
--------
Pure data parallel: batch 32 split 4 samples/core across 8 cores. The
routing statistics (mean/std of CLS-attention entropy over the full
batch) are needed by every core, but the stats input is tiny (32x4096
f32 = 512 KB), so instead of an all-reduce each core receives the FULL
cls_attention_map (rotated so its own 4 samples sit at rows 0..3 --
mean/std are permutation invariant) and computes the stats redundantly.
No collectives.

Per-core compute, all on-chip:
  * entropy: ACT ln + DVE multiply/reduce -> ent[32]
  * stats: DVE 32x32 transpose -> mean / unbiased var on one partition;
    level-2 gate in the squared domain (no sqrt):
      m2 = (ent<mean) & ((ent-mean)^2 > 2.25*var) & (var>1e-12), m1=1-m2
    broadcast to 128 partitions (gpsimd partition_broadcast).
  * expert select folded into the WEIGHTS: per sample the conv1 weights
    are scaled by m1 and the composite-y2 weights by m2 (exact: m is 0/1),
    so both experts accumulate into ONE PSUM tile and the epilogue is a
    single PSUM->SBUF copy.
  * wavelets as banded matmuls (sequence dim on partitions, D=768 on the
    free axis split 2x384 to fit PSUM banks), in float32r (single-pass
    PE mode, ~1.6e-4 rel err):
      y1 = db4 lowpass stride 2 (8 taps): 3 matmuls / 128-row chunk
      y2 = cascaded lowpass DIRECTLY from x via the composite 22-tap
           stride-4 filter g = (h upsampled by 2) conv h, exactly equal
           to dwt_lo(dwt_lo(x)) incl. zero-padding: 5 matmuls / chunk
  * loads on the sync (HWDGE) ring, stores + cls stats load on the
    gpsimd (SWDGE) ring so stores never head-of-line-block loads.

HBM traffic/core ~76 MB -> ~220 us roofline at ~350 GB/s/core;
PE ~170 us, DVE+ACT ~45 us each, hidden under DMA.
"""
import numpy as np

import concourse.bacc as bacc
import concourse.mybir as mybir
import concourse.tile as tile
from concourse.bass_utils import run_bass_kernel_spmd

AF = mybir.ActivationFunctionType
OP = mybir.AluOpType
DT = mybir.dt

B, NTOK, D = 32, 4097, 768
NPAT, L1, L2 = 4096, 2051, 1029
NCORES, SPC = 8, 4
NCH = 17          # output chunks per sample: 16 x 128 rows + 3-row tail
H = 2             # free-dim halves (2 x 384 <= one PSUM bank each)
HD = D // H

DEC_LO = np.array([-0.010597401785069032, 0.0328830116668852,
                   0.030841381835560764, -0.18703481171909309,
                   -0.027983769416859854, 0.6308807679298589,
                   0.7148465705529157, 0.2303778133088965], np.float64)
K8 = DEC_LO[::-1].copy()
G22 = np.zeros(22, np.float64)
for _j in range(8):
    for _k in range(8):
        G22[2 * _j + _k] += K8[_j] * K8[_k]

# runtime knobs (test harness pokes these)
TRACE = False
LAST_RESULT = None


def _band(filt, S, OFF, Dn, R=128):
    """W[m, t] = filt[128*Dn + m - S*t + OFF] (0 outside filter support)."""
    W = np.zeros((128, R), np.float32)
    m = np.arange(128)
    for t in range(R):
        idx = 128 * Dn + m - S * t + OFF
        ok = (idx >= 0) & (idx < len(filt))
        W[ok, t] = filt[idx[ok]]
    return W


def _weights():
    wc1 = np.concatenate([_band(K8, 2, 6, dn) for dn in (-1, 0, 1)], axis=1)
    wy2 = np.concatenate([_band(G22, 4, 18, dn) for dn in (-1, 0, 1, 2, 3)],
                         axis=1)
    return np.ascontiguousarray(wc1), np.ascontiguousarray(wy2)


def _build():
    import os
    mode = os.environ.get("KMODE", "full")  # full | conv (forced lvl1)
    nc = bacc.Bacc(None, target_bir_lowering=False)
    x = nc.declare_dram_parameter("x", [SPC, NTOK, D], DT.float32r,
                                  isOutput=False)
    cls = nc.declare_dram_parameter("cls", [B, NPAT], DT.float32,
                                    isOutput=False)
    wc1 = nc.declare_dram_parameter("wc1", [128, 3 * 128], DT.float32r,
                                    isOutput=False)
    wy2 = nc.declare_dram_parameter("wy2", [128, 5 * 128], DT.float32r,
                                    isOutput=False)
    out = nc.declare_dram_parameter("out", [SPC, 1 + L1, D], DT.float32,
                                    isOutput=True)
    mout = nc.declare_dram_parameter("mout", [2, B], DT.float32,
                                     isOutput=True)
    ones = nc.declare_dram_parameter("ones", [1, 128], DT.float32r,
                                     isOutput=False)

    with tile.TileContext(nc) as tc:
        with (
            tc.tile_pool(name="wp", bufs=1) as wp,
            tc.tile_pool(name="ws", bufs=2) as wsp,
            tc.tile_pool(name="ent", bufs=1) as entp,
            tc.tile_pool(name="xin", bufs=6) as xin,
            tc.tile_pool(name="ob", bufs=4) as obp,
            tc.tile_pool(name="otail", bufs=2) as otp,
            tc.tile_pool(name="crow", bufs=2) as crp,
            tc.tile_pool(name="tmp", bufs=2) as tmpp,
            tc.tile_pool(name="pa", bufs=4, space="PSUM") as pap,
        ):
            # ---------------- raw weights ----------------
            wc1t = wp.tile([128, 3 * 128], DT.float32r)
            wy2t = wp.tile([128, 5 * 128], DT.float32r)
            onest = wp.tile([1, 128], DT.float32r)

            # ---------------- entropy + routing stats ----------------
            mcols = entp.tile([128, 2 * B], DT.float32)
            if mode == "conv":
                nc.sync.dma_start(wc1t[:], wc1[:, :])
                nc.sync.dma_start(wy2t[:], wy2[:, :])
                nc.sync.dma_start(onest[:], ones[:, :])
                nc.vector.memset(mcols[:, 0:B], 1.0)
                nc.vector.memset(mcols[:, B:2 * B], 0.0)
            else:
                # cls reshaped to [128, 1024]: partition 4*b + q holds
                # quarter q of sample b -> 4x faster ACT/DVE big ops.
                # Loaded as the FIRST sync-ring DMA so it drains before the
                # x batches (FIFO per HWDGE ring).
                ent128 = entp.tile([128, NPAT // 4], DT.float32)
                csrc = cls[:, :].rearrange("b (q j) -> b q j", q=4)
                nc.sync.dma_start(ent128[:], csrc)
                nc.sync.dma_start(wc1t[:], wc1[:, :])
                nc.sync.dma_start(wy2t[:], wy2[:, :])
                nc.sync.dma_start(onest[:], ones[:, :])
                lnp = entp.tile([128, NPAT // 4], DT.float32)
                bias9 = entp.tile([128, 1], DT.float32)
                nc.vector.memset(bias9[:], 1e-9)
                # dependency-free warm-up Ln so the ACT table set loads
                # while the cls DMA is still in flight
                warm = entp.tile([128, 1], DT.float32)
                nc.scalar.activation(warm[:], bias9[:], AF.Ln, bias=bias9[:])
                nc.scalar.activation(lnp[:], ent128[:], AF.Ln, bias=bias9[:])
                # fused (-1/ln2 * p) * ln(p+1e-9) + row-sum -> [128, 1]
                scr = entp.tile([128, NPAT // 4], DT.float32)
                part = entp.tile([128, 1], DT.float32)
                nc.vector.scalar_tensor_tensor(scr[:], ent128[:],
                                               -1.4426950408889634,
                                               lnp[:], op0=OP.mult,
                                               op1=OP.mult,
                                               accum_out=part[:])
                # partition -> free flatten via a tiny DMA, then sum the 4
                # quarters per sample along the free axis
                prow = entp.tile([1, 128], DT.float32)
                nc.scalar.dma_start(prow[:], part[:])
                entrow = entp.tile([1, 32], DT.float32)
                prview = prow[0:1, :].rearrange("o (b q) -> o b q", q=4)
                nc.vector.tensor_reduce(entrow[:], prview,
                                        axis=mybir.AxisListType.X, op=OP.add)
                erow = entrow[0:1, 0:B]
                ssum = entp.tile([1, 1], DT.float32)
                nc.vector.tensor_reduce(ssum[:], erow,
                                        axis=mybir.AxisListType.X, op=OP.add)
                mean = entp.tile([1, 1], DT.float32)
                nc.vector.tensor_scalar_mul(mean[:], ssum[:], 1.0 / B)
                dev = entp.tile([1, B], DT.float32)
                nc.vector.tensor_scalar_sub(dev[:], erow, mean[0:1, 0:1])
                dev2 = entp.tile([1, B], DT.float32)
                nc.vector.tensor_mul(dev2[:], dev[:], dev[:])
                ss2 = entp.tile([1, 1], DT.float32)
                nc.vector.tensor_reduce(ss2[:], dev2[:],
                                        axis=mybir.AxisListType.X, op=OP.add)
                var = entp.tile([1, 1], DT.float32)
                nc.vector.tensor_scalar_mul(var[:], ss2[:], 1.0 / (B - 1))
                var225 = entp.tile([1, 1], DT.float32)
                nc.vector.tensor_scalar_mul(var225[:], var[:], 2.25)
                c1 = entp.tile([1, B], DT.float32)
                nc.vector.tensor_scalar(c1[:], dev[:], 0.0, None,
                                        op0=OP.is_lt)
                c2 = entp.tile([1, B], DT.float32)
                nc.vector.tensor_scalar(c2[:], dev2[:], var225[0:1, 0:1],
                                        None, op0=OP.is_gt)
                c3 = entp.tile([1, 1], DT.float32)
                nc.vector.tensor_scalar(c3[:], var[:], 1e-12, None,
                                        op0=OP.is_gt)
                m2a = entp.tile([1, B], DT.float32)
                nc.vector.tensor_mul(m2a[:], c1[:], c2[:])
                mpack = entp.tile([1, 2 * B], DT.float32r)
                m2row = entp.tile([1, B], DT.float32)
                nc.vector.tensor_scalar_mul(m2row[:], m2a[:], c3[0:1, 0:1])
                nc.vector.tensor_scalar_mul(
                    mpack[:, B:2 * B], m2row[:], 1.0)
                nc.vector.tensor_scalar(mpack[:, 0:B], m2row[:], -1.0, 1.0,
                                        op0=OP.mult, op1=OP.add)
                # exact broadcast to 128 partitions via PE: ones^T @ mpack
                mps = pap.tile([128, 2 * B], DT.float32, tag="pa")
                nc.tensor.matmul(mps[:], onest[:], mpack[:],
                                 start=True, stop=True)
                nc.vector.tensor_copy(mcols[:], mps[:])
                nc.gpsimd.dma_start(mout[0:1, :], m2row[:])
                nc.gpsimd.dma_start(mout[1:2, :], entrow[0:1, 0:B])

            # ---------------- main per-sample pipeline ----------------
            for b in range(SPC):
                m1c = mcols[:, b:b + 1]
                m2c = mcols[:, B + b:B + b + 1]

                # expert select folded into the weights (m is exactly 0/1)
                # for b >= 1; sample 0 runs both experts unscaled into two
                # PSUM tiles and selects in the epilogue, so its matmuls
                # need only the raw weights (PE starts ~15us earlier)
                fused = True  # hybrid b==0 fast-start measured slower; keep fused
                if fused:
                    wc1s = wsp.tile([128, 3 * 128], DT.float32r, tag="wc1s")
                    nc.scalar.mul(wc1s[:], wc1t[:], m1c)
                    wy2s = wsp.tile([128, 5 * 128], DT.float32r, tag="wy2s")
                    nc.vector.tensor_scalar_mul(wy2s[:], wy2t[:], m2c)
                else:
                    wc1s, wy2s = wc1t, wy2t

                # CLS token passthrough
                crow = crp.tile([1, D], DT.float32r)
                nc.sync.dma_start(crow[:], x[b, 0:1, :])
                nc.gpsimd.dma_start(out[b, 0:1, :],
                                    crow[:].bitcast(DT.float32))

                # patch loads: 8 DMAs of 4 chunks each
                xts = []
                for q in range(8):
                    xt = xin.tile([128, 4, D], DT.float32r)
                    src = x[b, 1 + 512 * q: 1 + 512 * (q + 1), :].rearrange(
                        "(blk p) d -> p blk d", p=128)
                    nc.sync.dma_start(xt[:], src)
                    xts.append(xt)

                def xch(i, h):
                    q, r = divmod(i, 4)
                    return xts[q][:, r, h * HD:(h + 1) * HD]

                ob = None
                for jc in range(NCH):
                    R = 128 if jc < 16 else 3
                    qo, blk = divmod(jc, 4)
                    # matmul schedule for this chunk: conv1 and y2;
                    # fused: all into one PSUM tile (weights pre-scaled)
                    # unfused (b==0): separate PSUM tiles + epilogue select
                    s1, s2 = [], []
                    if jc < 16:
                        for dn in (-1, 0, 1):
                            if 0 <= 2 * jc + dn < 32:
                                s1.append((wc1s, dn, 2 * jc + dn, R))
                    else:
                        s1.append((wc1s, -1, 31, R))
                    if jc < 8:
                        for dn in (-1, 0, 1, 2, 3):
                            if 0 <= 4 * jc + dn < 32:
                                s2.append((wy2s, dn, 4 * jc + dn, 128))
                    elif jc == 8:
                        # tail y2 rows 0..4; full-128 weight slice: columns
                        # past t=4 are all-zero so rows 5..127 get +0
                        s2.append((wy2s, -1, 31, 128))

                    pa = pap.tile([128, H, 512], DT.float32)
                    pb = None
                    if fused:
                        groups = [(pa, s1 + s2)]
                    else:
                        groups = [(pa, s1)]
                        if s2:
                            pb = pap.tile([128, H, 512], DT.float32,
                                          tag="pa")
                            groups.append((pb, s2))
                    for ps, sched in groups:
                        for h in range(H):
                            for k, (wt, dn, i, r) in enumerate(sched):
                                nc.tensor.matmul(
                                    ps[0:r, h, 0:HD],
                                    wt[:, (dn + 1) * 128:(dn + 1) * 128 + r],
                                    xch(i, h),
                                    start=(k == 0),
                                    stop=(k == len(sched) - 1))

                    # epilogue: PSUM->SBUF (fused: plain copy; sample 0:
                    # final = m1*PA + m2*PB select), alternating engines
                    if jc < 16:
                        if blk == 0:
                            ob = obp.tile([128, 4, H, HD], DT.float32)
                        dst = ob[0:R, blk]
                    else:
                        otail = otp.tile([3, H, HD], DT.float32)
                        dst = otail[0:R]
                    if fused:
                        if jc % 2:
                            nc.vector.tensor_copy(dst, pa[0:R, :, 0:HD])
                        else:
                            nc.scalar.copy(dst, pa[0:R, :, 0:HD])
                    elif pb is not None:
                        tmp = tmpp.tile([128, H, HD], DT.float32)
                        nc.scalar.mul(tmp[0:R], pb[0:R, :, 0:HD], m2c[0:R])
                        nc.vector.scalar_tensor_tensor(
                            dst, pa[0:R, :, 0:HD], m1c[0:R], tmp[0:R],
                            op0=OP.mult, op1=OP.add)
                    else:
                        if jc % 2:
                            nc.vector.tensor_scalar_mul(
                                dst, pa[0:R, :, 0:HD], m1c[0:R])
                        else:
                            nc.scalar.mul(dst, pa[0:R, :, 0:HD], m1c[0:R])

                    # flush output batches (SWDGE ring; never blocks loads)
                    if jc < 16 and blk == 3:
                        d_ap = out[b, 1 + 512 * qo: 1 + 512 * (qo + 1),
                                   :].rearrange("(blk p) d -> p blk d", p=128)
                        nc.gpsimd.dma_start(d_ap, ob[:])
                    elif jc == 16:
                        nc.gpsimd.dma_start(out[b, 2049:2052, :], dst)
    nc.compile()
    return nc


_NC = None


def _get_nc():
    global _NC
    if _NC is None:
        _NC = _build()
    return _NC


def kernel(x, cls_attention_map):
    global LAST_RESULT
    x = np.ascontiguousarray(np.asarray(x, dtype=np.float32))
    cls = np.ascontiguousarray(np.asarray(cls_attention_map,
                                          dtype=np.float32))
    wc1, wy2 = _weights()
    in_maps = []
    for c in range(NCORES):
        in_maps.append({
            "x": np.ascontiguousarray(x[SPC * c: SPC * (c + 1)]),
            "cls": np.ascontiguousarray(np.roll(cls, -SPC * c, axis=0)),
            "wc1": wc1,
            "wy2": wy2,
            "ones": np.ones((1, 128), np.float32),
        })
    res = run_bass_kernel_spmd(_get_nc(), in_maps,
                               core_ids=list(range(NCORES)), trace=TRACE)
    LAST_RESULT = res

    fin = np.empty((B, 1 + L1, D), np.float32)
    lvl = np.empty(B, np.int64)
    for c in range(NCORES):
        r = res.results[c]
        fin[SPC * c: SPC * (c + 1)] = r["out"]
        m2v = np.asarray(r["mout"]).reshape(2, B)[0, 0:SPC]
        lvl[SPC * c: SPC * (c + 1)] = 1 + (m2v > 0.5).astype(np.int64)
    out_len = np.where(lvl == 1, L1, L2)
    mask = np.arange(1 + L1)[None, :] < (1 + out_len)[:, None]
    return fin, mask
